# revision 19
# baseline (speedup 1.0000x reference)
"""Causal self-attention (B=2,T=2048,C=1024,H=16) on 8 trn2 cores.

Sharding: core = (batch b, head-group g); b = core//4, g = core%4.
Each core computes attention for 4 heads of one batch plus the
row-parallel slice of c_proj; host sums the 4 partial projections per
batch and adds b_proj.

All matmul operands are bf16 (hosts pre-casts inputs); accumulation is
fp32 in PSUM.  The emission order software-pipelines the program:
window j's attention (Act-bound exp stream) is interleaved with window
j+1's QKV projection and window j-1's c_proj on the PE so the tensor
engine never idles and stays at full p-state.
"""

import os
from collections import deque

import numpy as np
import ml_dtypes

import concourse.bass as bass
import concourse.mybir as mybir
import concourse.tile as tile
from concourse.bass import ts, ds
from concourse.bass_utils import run_bass_kernel_spmd
from concourse.vector_clock import ScopedClock

# ---------------------------------------------------------------------------
# Workaround: walrus CoreV3 rejects >2 sem waits on one instruction; the
# TileContext exit drain accumulates one wait per outstanding proc.  Split
# them across single-wait sync nops before the drain.
# ---------------------------------------------------------------------------


def _patched_drain_and_barrier(self, tick_clock, wait_clock):
    nc = self.nc
    probe = mybir.InstNoOp(name=nc.get_next_instruction_name(), ins=[], outs=[])
    probe.engine = mybir.EngineType.SP
    wait_clock.add_sem_waits(probe, ScopedClock({None: tick_clock.global_clock}))
    waits = list(probe.sync_info.on_wait) if probe.sync_info else []
    for w in waits:
        n = nc.sync.nop(nofuse=True, hint="drain_wait_split")
        n.ins.sync_info = mybir.SyncInfo(on_wait=[w], on_update=[])
    nc.sync.drain()
    nc.all_engine_barrier()
    assert self.sems is not None
    popped = nc._tile_sem_poison_stack.pop()
    assert popped is self._sem_poison
    nc.clear_and_free_semaphores(list(self.sems.allocated().values()))
    nc.all_engine_barrier()


tile.TileContext._drain_and_barrier = _patched_drain_and_barrier

_DMA_INSTS = (
    mybir.InstCollectiveCompute,
)


def split_excess_waits(nc):
    """walrus CoreV3 encodes at most 1 sem wait per compute instruction
    (2 on EventSemaphore); hoist extras onto same-engine nops."""
    for fn in nc.m.functions:
        for bb in fn.blocks:
            insts = bb.instructions
            new_list = []
            changed = False
            for inst in insts:
                si = inst.sync_info
                cap = 2 if isinstance(inst, mybir.InstEventSemaphore) else 1
                if (
                    si is not None
                    and not isinstance(inst, _DMA_INSTS)
                    and len(si.on_wait) > cap
                ):
                    waits = list(si.on_wait)
                    extra, keep = waits[:-cap], waits[-cap:]
                    for w in extra:
                        nop = mybir.InstNoOp(
                            name=nc.get_next_instruction_name(), ins=[], outs=[]
                        )
                        nop.engine = inst.engine
                        nop.sync_info = mybir.SyncInfo(on_wait=[w], on_update=[])
                        nc.register_instruction(nop)
                        new_list.append(nop)
                    inst.sync_info = mybir.SyncInfo(
                        on_wait=keep, on_update=list(si.on_update)
                    )
                    changed = True
                new_list.append(inst)
            if changed:
                bb.instructions = new_list

# ---------------------------------------------------------------------------

B, T, C, H, HD = 2, 2048, 1024, 16, 64
NCORES, GROUPS = 8, 4
CL = C // GROUPS          # 256 channels (4 heads) per core
HPC = H // GROUPS         # 4 heads per core
F32 = mybir.dt.float32
BF = mybir.dt.bfloat16

QT = 512                  # q window (free dim of S^T tiles)
NQW = T // QT             # 4 q windows
NKT = T // 128            # 16 k tiles of 128

REPS = int(os.environ.get("KREPS", "1"))


def build_nc():
    nc = bass.Bass()
    # chunkp packs [x(t<512) | Wqk | Wv] per channel row so phase A's whole
    # working set arrives in 8 DMAs; xhi carries x for t>=512 per window.
    chunkp = nc.dram_tensor("chunkp", [C, QT + 2 * CL + CL], BF, kind="ExternalInput")
    xhi = nc.dram_tensor("xhi", [C, 3 * QT], BF, kind="ExternalInput")
    wpT = nc.dram_tensor("wpT", [CL, C], BF, kind="ExternalInput")
    bqk = nc.dram_tensor("bqk", [2 * CL], F32, kind="ExternalInput")
    bvb = nc.dram_tensor("bvb", [128, CL], F32, kind="ExternalInput")
    m0 = nc.dram_tensor("m0", [128, 128], BF, kind="ExternalInput")
    onesf = nc.dram_tensor("onesf", [1, 64], BF, kind="ExternalInput")
    outp = nc.dram_tensor("outp", [T, C], BF, kind="ExternalOutput")
    DEBUG = os.environ.get("KDEBUG") == "1"
    if DEBUG:
        dbg_qkT = nc.dram_tensor("dbg_qkT", [128, 4, T], BF, kind="ExternalOutput")
        dbg_vaug = nc.dram_tensor("dbg_vaug", [128, NKT, HPC, HD + 1], BF, kind="ExternalOutput")
        dbg_yT = nc.dram_tensor("dbg_yT", [128, 2, T], BF, kind="ExternalOutput")

    AF = mybir.ActivationFunctionType
    OP = mybir.AluOpType

    with tile.TileContext(nc) as tc:
        from contextlib import ExitStack
        with ExitStack() as ctx:
            persist = ctx.enter_context(tc.tile_pool(name="persist", bufs=1))
            qkvin = ctx.enter_context(tc.tile_pool(name="qkvin", bufs=1))
            work = ctx.enter_context(tc.tile_pool(name="work", bufs=4))
            bcast = ctx.enter_context(tc.tile_pool(name="bcast", bufs=2))
            outsb = ctx.enter_context(tc.tile_pool(name="outsb", bufs=2))
            psS = ctx.enter_context(tc.tile_pool(name="psS", bufs=2, space="PSUM"))
            psQ = ctx.enter_context(tc.tile_pool(name="psQ", bufs=2, space="PSUM"))
            psY = ctx.enter_context(tc.tile_pool(name="psY", bufs=2, space="PSUM"))

            # persistent tensors
            qkT = persist.tile([128, 4, T], BF)           # o-tiles: q01 q23 k01 k23
            vaug = persist.tile([128, NKT, HPC, HD + 1], BF)
            yT = persist.tile([128, 2, T], BF)            # heads stacked on (part, chunk)
            wp_s = persist.tile([128, 2, C], BF)
            m0_s = persist.tile([128, 128], BF)
            bq_s = persist.tile([128, 4], F32)
            bv_s = persist.tile([128, CL], F32)
            ones_f = persist.tile([1, 64], BF)

            PK = QT + 2 * CL + CL      # 1280 packed columns per chunk
            chunk_s = qkvin.tile([128, 8, PK], BF)
            xhi_s = qkvin.tile([128, 3, 8, QT], BF)

            def xw(cc, w):
                """x^T slice [128, QT] for chunk cc, window w."""
                if w == 0:
                    return chunk_s[:, cc, 0:QT]
                return xhi_s[:, w - 1, cc, :]

            wqk_s = chunk_s[:, :, QT:QT + 2 * CL]
            wv_s = chunk_s[:, :, QT + 2 * CL:PK]

            chunk_r = chunkp.rearrange("(cc p) f -> p cc f", p=128)
            xhi_r = xhi.rearrange("(cc p) (w t) -> p cc w t", p=128, t=QT)
            for cc in range(8):
                nc.sync.dma_start(out=chunk_s[:, cc, :], in_=chunk_r[:, cc, :])
            for w in range(3):
                nc.sync.dma_start(out=xhi_s[:, w, :, :], in_=xhi_r[:, :, w, :])
            nc.sync.dma_start(out=wp_s, in_=wpT.rearrange("(cc p) o -> p cc o", p=128))
            # small loads ride the otherwise-idle Act queue
            nc.scalar.dma_start(out=m0_s, in_=m0[:, :])
            nc.scalar.dma_start(out=bq_s, in_=bqk.rearrange("(o p) -> p o", p=128))
            nc.scalar.dma_start(out=bv_s, in_=bvb[:, :])
            nc.scalar.dma_start(out=ones_f, in_=onesf[:, :])
            nc.vector.memset(vaug[:, :, :, HD:HD + 1], 1.0)

            for _rep in range(REPS):
                # ============ phase A: window-0 q/k + v, cc-outer ============
                psA = [psS.tile([128, 2, QT], F32, tag="s", name=f"psA{_i}") for _i in range(2)]
                psV = [psQ.tile([128, QT], F32, tag="q", name=f"psV{_i}") for _i in range(2)]
                for cc in range(8):
                    for o in range(4):
                        nc.tensor.matmul(
                            psA[o // 2][:, o % 2, :],
                            lhsT=wqk_s[:, cc, ts(o, 128)],
                            rhs=xw(cc, 0),
                            start=(cc == 0),
                            stop=(cc == 7),
                        )
                    for tt in range(4):
                        # PSUM start resets the whole bank: only the first
                        # group in a shared bank may assert it; the sibling
                        # half accumulates onto the zeros that reset left.
                        nc.tensor.matmul(
                            psV[tt // 2][:, ts(tt % 2, CL)],
                            lhsT=xw(cc, 0)[:, ts(tt, 128)],
                            rhs=wv_s[:, cc, :],
                            start=(cc == 0 and tt % 2 == 0),
                            stop=(cc == 7),
                            skip_group_check=True,
                        )
                # evict in first-use order (heads 0/1 read o0+o2 first) and
                # split across DVE/Act so the two chains run in parallel.
                for o in (0, 2, 1, 3):
                    sc = 0.125 if o < 2 else 1.0
                    if o >= 2:
                        nc.scalar.activation(
                            out=qkT[:, o, 0:QT],
                            in_=psA[o // 2][:, o % 2, :],
                            func=AF.Identity,
                            scale=sc,
                            bias=bq_s[:, o:o + 1],
                        )
                    else:
                        nc.vector.tensor_scalar(
                            out=qkT[:, o, 0:QT],
                            in0=psA[o // 2][:, o % 2, :],
                            scalar1=sc,
                            scalar2=bq_s[:, o:o + 1],
                            op0=OP.mult,
                            op1=OP.add,
                        )
                for tt in range(4):
                    nc.vector.tensor_add(
                        out=vaug[:, tt, :, 0:HD],
                        in0=psV[tt // 2][:, ts(tt % 2, CL)].rearrange(
                            "p (h d) -> p h d", h=HPC),
                        in1=bv_s.rearrange("p (h d) -> p h d", h=HPC),
                    )

                # ============ filler machinery ============
                crit = deque()   # next window's q/k + v (must flush at boundary)
                lazy = deque()   # c_proj of finished windows

                def pump(n, lazy_ok=False):
                    for _ in range(n):
                        if crit:
                            crit.popleft()()
                        elif lazy and lazy_ok:
                            lazy.popleft()()
                        else:
                            return

                def flush_crit():
                    while crit:
                        crit.popleft()()

                def enqueue_qkv(jw):
                    t0 = jw * QT
                    for o in range(4):
                        def qk_group(o=o, t0=t0):
                            st = {"ps": None}
                            def mm(cc, st=st, o=o, t0=t0):
                                if cc == 0:
                                    st["ps"] = psQ.tile([128, QT], F32, tag="q", name="qkfill")
                                nc.tensor.matmul(
                                    st["ps"],
                                    lhsT=wqk_s[:, cc, ts(o, 128)],
                                    rhs=xw(cc, t0 // QT),
                                    start=(cc == 0),
                                    stop=(cc == 7),
                                )
                                if cc == 7:
                                    sc = 0.125 if o < 2 else 1.0
                                    nc.vector.tensor_scalar(
                                        out=qkT[:, o, ds(t0, QT)],
                                        in0=st["ps"],
                                        scalar1=sc,
                                        scalar2=bq_s[:, o:o + 1],
                                        op0=OP.mult,
                                        op1=OP.add,
                                    )
                            return mm
                        g = qk_group()
                        for cc in range(8):
                            crit.append(lambda cc=cc, g=g: g(cc))
                    for tp in range(2):
                        def v_group(tp=tp, jw=jw):
                            st = {"ps": None}
                            def mm(cc, i, st=st, tp=tp, jw=jw):
                                tt = jw * 4 + tp * 2 + i
                                if cc == 0 and i == 0:
                                    st["ps"] = psQ.tile([128, QT], F32, tag="q", name="qkfill")
                                nc.tensor.matmul(
                                    st["ps"][:, ts(i, CL)],
                                    lhsT=xw(cc, tt // 4)[:, ts(tt % 4, 128)],
                                    rhs=wv_s[:, cc, :],
                                    start=(cc == 0 and i == 0),
                                    stop=(cc == 7),
                                    skip_group_check=True,
                                )
                                if cc == 7:
                                    nc.vector.tensor_add(
                                        out=vaug[:, tt, :, 0:HD],
                                        in0=st["ps"][:, ts(i, CL)].rearrange(
                                            "p (h d) -> p h d", h=HPC),
                                        in1=bv_s.rearrange("p (h d) -> p h d", h=HPC),
                                    )
                            return mm
                        g = v_group()
                        for cc in range(8):
                            for i in range(2):
                                crit.append(lambda cc=cc, i=i, g=g: g(cc, i))

                def enqueue_cproj(jw):
                    for tl in range(4):
                        tt = jw * 4 + tl
                        def cp_group(tt=tt, jw=jw):
                            st = {"ob": None, "ps": None}
                            def mm(nn_, c2, st=st, tt=tt, jw=jw):
                                if nn_ == 0 and c2 == 0:
                                    st["ob"] = outsb.tile([128, C], BF, tag="ob", name="obt")
                                if c2 == 0:
                                    st["ps"] = psQ.tile([128, QT], F32, tag="q", name="qkfill")
                                nc.tensor.matmul(
                                    st["ps"],
                                    lhsT=yT[:, c2, ts(tt, 128)],
                                    rhs=wp_s[:, c2, ts(nn_, 512)],
                                    start=(c2 == 0),
                                    stop=(c2 == 1),
                                )
                                if c2 == 1:
                                    # alternate evictions DVE/Act so the tail
                                    # is not serialized on one engine
                                    if jw < NQW - 1 or (tt + nn_) % 2 == 0:
                                        nc.vector.tensor_copy(
                                            out=st["ob"][:, ts(nn_, 512)], in_=st["ps"])
                                    else:
                                        nc.scalar.copy(
                                            out=st["ob"][:, ts(nn_, 512)], in_=st["ps"])
                                if nn_ == 1 and c2 == 1:
                                    nc.sync.dma_start(
                                        out=outp[ts(tt, 128), :], in_=st["ob"])
                            return mm
                        g = cp_group()
                        for nn_ in range(2):
                            for c2 in range(2):
                                lazy.append(lambda nn_=nn_, c2=c2, g=g: g(nn_, c2))

                # ============ pipelined windows ============
                for j in range(NQW):
                    # c_proj fillers are held back until the last window,
                    # whose exp stream leaves the PE otherwise underfed.
                    lazy_ok = (j == NQW - 1)
                    if j + 1 < NQW:
                        enqueue_qkv(j + 1)
                    nkt = 4 * (j + 1)
                    for h in range(HPC):
                        hp, w = h // 2, h % 2
                        pl = 64 * w
                        psy = psY.tile([128, QT], F32, tag="y")
                        pending = []
                        for g in range(nkt // 2):
                            pt = work.tile([128, 2, QT], BF, tag="pt")
                            pss = psS.tile([128, 2, QT], F32, tag="s")
                            cur = []
                            for i in range(2):
                                kt = 2 * g + i
                                m = kt - 4 * j
                                q_lo = m * 128 if m >= 0 else 0
                                n = QT - q_lo
                                nc.tensor.matmul(
                                    pss[:, i, q_lo:QT],
                                    lhsT=qkT[pl:pl + 64, 2 + hp, ts(kt, 128)],
                                    rhs=qkT[pl:pl + 64, hp, ds(j * QT + q_lo, n)],
                                    start=True,
                                    stop=True,
                                )
                                cur.append((pt, i, kt, q_lo))
                            pump(3, lazy_ok)
                            # one exp per group: the [mq, q_lo_i) slice of a
                            # diagonal tile exps stale-but-bounded PSUM data
                            # that the PV matmul never reads.
                            mq = min(q_lo for (_, _, _, q_lo) in cur)
                            nc.scalar.activation(
                                out=pt[:, :, mq:QT], in_=pss[:, :, mq:QT],
                                func=AF.Exp)
                            for (ptile, i, kt, q_lo) in cur:
                                if kt - 4 * j >= 0:
                                    nc.vector.tensor_mul(
                                        out=ptile[:, i, ds(q_lo, 128)],
                                        in0=ptile[:, i, ds(q_lo, 128)],
                                        in1=m0_s,
                                    )
                            pending.append(cur)
                            if len(pending) > 2:
                                for (ptile, i, kt, q_lo) in pending.pop(0):
                                    nc.tensor.matmul(
                                        psy[0:65, q_lo:QT],
                                        lhsT=vaug[:, kt, h, :],
                                        rhs=ptile[:, i, q_lo:QT],
                                        start=(kt == 0),
                                        stop=(kt == nkt - 1),
                                    )
                                pump(1, lazy_ok)
                        for grp in pending:
                            for (ptile, i, kt, q_lo) in grp:
                                nc.tensor.matmul(
                                    psy[0:65, q_lo:QT],
                                    lhsT=vaug[:, kt, h, :],
                                    rhs=ptile[:, i, q_lo:QT],
                                    start=(kt == 0),
                                    stop=(kt == nkt - 1),
                                )
                            pump(1, lazy_ok)
                        # normalize: y^T = y_aug^T * (1/denom); denom bcast on Pool
                        rc = bcast.tile([1, QT], BF, tag="rc")
                        with nc.allow_low_precision(reason="1/denom in bf16, ~0.2% rel"):
                            nc.vector.reciprocal(out=rc, in_=psy[64:65, :])
                        nc.tensor.matmul(
                            psy[64:128, :],
                            lhsT=ones_f,
                            rhs=rc,
                            start=True,
                            stop=True,
                            skip_group_check=True,
                        )
                        dn = bcast.tile([64, QT], BF, tag="dn")
                        with nc.allow_low_precision(reason="1/denom copy in bf16"):
                            nc.vector.tensor_copy(out=dn, in_=psy[64:128, :])
                        nc.vector.tensor_mul(
                            out=yT[pl:pl + 64, hp, ts(j, QT)],
                            in0=psy[0:64, :],
                            in1=dn,
                        )
                        pump(2, lazy_ok)
                    enqueue_cproj(j)
                    flush_crit()
                while crit or lazy:
                    pump(1, True)
            if DEBUG:
                nc.sync.dma_start(out=dbg_qkT[:, :, :], in_=qkT)
                nc.sync.dma_start(out=dbg_vaug[:, :, :, :], in_=vaug)
                nc.sync.dma_start(out=dbg_yT[:, :, :], in_=yT)

    split_excess_waits(nc)
    return nc


_NC_CACHE = None


def _get_nc():
    global _NC_CACHE
    if _NC_CACHE is None:
        _NC_CACHE = build_nc()
    return _NC_CACHE


def make_in_maps(x, W_attn, b_attn, W_proj):
    bf16 = ml_dtypes.bfloat16
    x = np.asarray(x, np.float32)
    W_attn = np.asarray(W_attn, np.float32)
    b_attn = np.asarray(b_attn, np.float32)
    W_proj = np.asarray(W_proj, np.float32)
    m0 = np.triu(np.ones((128, 128), np.float32))  # keep q >= k
    in_maps = []
    for core in range(NCORES):
        b, g = core // GROUPS, core % GROUPS
        qr = slice(g * CL, (g + 1) * CL)
        kr = slice(C + g * CL, C + (g + 1) * CL)
        vr = slice(2 * C + g * CL, 2 * C + (g + 1) * CL)
        wqk = np.concatenate([W_attn[qr], W_attn[kr]], axis=0)      # [512, 1024]
        xTb = x[b].T                                        # [C, T]
        packed = np.concatenate(
            [xTb[:, :512], wqk.T, W_attn[vr].T], axis=1)    # [C, 1280]
        in_maps.append({
            "chunkp": np.ascontiguousarray(packed).astype(bf16),
            "xhi": np.ascontiguousarray(xTb[:, 512:]).astype(bf16),
            "wpT": np.ascontiguousarray(W_proj[:, g * CL:(g + 1) * CL].T).astype(bf16),
            "bqk": np.concatenate([b_attn[qr] / 8.0, b_attn[kr]]).astype(np.float32),
            "bvb": np.broadcast_to(b_attn[vr], (128, CL)).astype(np.float32).copy(),
            "m0": m0.astype(bf16),
            "onesf": np.ones((1, 64), bf16),
        })
    return in_maps


def kernel(x, W_attn, b_attn, W_proj, b_proj, **_unused):
    nc = _get_nc()
    in_maps = make_in_maps(x, W_attn, b_attn, W_proj)
    res = run_bass_kernel_spmd(nc, in_maps, core_ids=list(range(NCORES)))
    out = np.zeros((B, T, C), np.float32)
    for core in range(NCORES):
        out[core // GROUPS] += res.results[core]["outp"].astype(np.float32)
    out += np.asarray(b_proj, np.float32)[None, None, :]
    return out


# revision 23
# speedup vs baseline: 1.2544x; 1.2544x over previous
"""Causal self-attention (B=2,T=2048,C=1024,H=16) on 8 trn2 cores.

Sharding: core = (batch b, head-group g); b = core//4, g = core%4.
Each core computes attention for 4 heads of one batch plus the
row-parallel slice of c_proj; host sums the 4 partial projections per
batch and adds b_proj.

All matmul operands are bf16 (hosts pre-casts inputs); accumulation is
fp32 in PSUM.  The emission order software-pipelines the program:
window j's attention (Act-bound exp stream) is interleaved with window
j+1's QKV projection and window j-1's c_proj on the PE so the tensor
engine never idles and stays at full p-state.
"""

import os
from collections import deque

import numpy as np
import ml_dtypes

import concourse.bass as bass
import concourse.mybir as mybir
import concourse.tile as tile
from concourse.bass import ts, ds
from concourse.bass_utils import run_bass_kernel_spmd
from concourse.vector_clock import ScopedClock

# ---------------------------------------------------------------------------
# Workaround: walrus CoreV3 rejects >2 sem waits on one instruction; the
# TileContext exit drain accumulates one wait per outstanding proc.  Split
# them across single-wait sync nops before the drain.
# ---------------------------------------------------------------------------


def _patched_drain_and_barrier(self, tick_clock, wait_clock):
    nc = self.nc
    probe = mybir.InstNoOp(name=nc.get_next_instruction_name(), ins=[], outs=[])
    probe.engine = mybir.EngineType.SP
    wait_clock.add_sem_waits(probe, ScopedClock({None: tick_clock.global_clock}))
    waits = list(probe.sync_info.on_wait) if probe.sync_info else []
    for w in waits:
        n = nc.sync.nop(nofuse=True, hint="drain_wait_split")
        n.ins.sync_info = mybir.SyncInfo(on_wait=[w], on_update=[])
    nc.sync.drain()
    nc.all_engine_barrier()
    assert self.sems is not None
    popped = nc._tile_sem_poison_stack.pop()
    assert popped is self._sem_poison
    nc.clear_and_free_semaphores(list(self.sems.allocated().values()))
    nc.all_engine_barrier()


tile.TileContext._drain_and_barrier = _patched_drain_and_barrier

_DMA_INSTS = (
    mybir.InstCollectiveCompute,
)


def split_excess_waits(nc):
    """walrus CoreV3 encodes at most 1 sem wait per compute instruction
    (2 on EventSemaphore); hoist extras onto same-engine nops."""
    for fn in nc.m.functions:
        for bb in fn.blocks:
            insts = bb.instructions
            new_list = []
            changed = False
            for inst in insts:
                si = inst.sync_info
                cap = 2 if isinstance(inst, mybir.InstEventSemaphore) else 1
                if (
                    si is not None
                    and not isinstance(inst, _DMA_INSTS)
                    and len(si.on_wait) > cap
                ):
                    waits = list(si.on_wait)
                    extra, keep = waits[:-cap], waits[-cap:]
                    for w in extra:
                        nop = mybir.InstNoOp(
                            name=nc.get_next_instruction_name(), ins=[], outs=[]
                        )
                        nop.engine = inst.engine
                        nop.sync_info = mybir.SyncInfo(on_wait=[w], on_update=[])
                        nc.register_instruction(nop)
                        new_list.append(nop)
                    inst.sync_info = mybir.SyncInfo(
                        on_wait=keep, on_update=list(si.on_update)
                    )
                    changed = True
                new_list.append(inst)
            if changed:
                bb.instructions = new_list

# ---------------------------------------------------------------------------

B, T, C, H, HD = 2, 2048, 1024, 16, 64
NCORES, GROUPS = 8, 4
CL = C // GROUPS          # 256 channels (4 heads) per core
HPC = H // GROUPS         # 4 heads per core
F32 = mybir.dt.float32
BF = mybir.dt.bfloat16

QT = 512                  # q window (free dim of S^T tiles)
NQW = T // QT             # 4 q windows
NKT = T // 128            # 16 k tiles of 128

REPS = int(os.environ.get("KREPS", "1"))


def build_nc():
    nc = bass.Bass()
    # chunkp packs [x(t<512) | Wqk | Wv] per channel row so phase A's whole
    # working set arrives in 8 DMAs; xhi carries x for t>=512 per window.
    chunkp = nc.dram_tensor("chunkp", [C, QT + 2 * CL + CL], BF, kind="ExternalInput")
    xhi = nc.dram_tensor("xhi", [C, 3 * QT], BF, kind="ExternalInput")
    wpT = nc.dram_tensor("wpT", [CL, C], BF, kind="ExternalInput")
    bqk = nc.dram_tensor("bqk", [2 * CL], F32, kind="ExternalInput")
    bvb = nc.dram_tensor("bvb", [128, CL], F32, kind="ExternalInput")
    m0 = nc.dram_tensor("m0", [128, 128], BF, kind="ExternalInput")
    onesf = nc.dram_tensor("onesf", [1, 64], BF, kind="ExternalInput")
    outp = nc.dram_tensor("outp", [T, C], BF, kind="ExternalOutput")
    DEBUG = os.environ.get("KDEBUG") == "1"
    if DEBUG:
        dbg_qkT = nc.dram_tensor("dbg_qkT", [128, 4, T], BF, kind="ExternalOutput")
        dbg_vaug = nc.dram_tensor("dbg_vaug", [128, NKT, HPC, HD + 1], BF, kind="ExternalOutput")
        dbg_yT = nc.dram_tensor("dbg_yT", [128, 2, T], BF, kind="ExternalOutput")

    AF = mybir.ActivationFunctionType
    OP = mybir.AluOpType

    with tile.TileContext(nc) as tc:
        from contextlib import ExitStack
        with ExitStack() as ctx:
            persist = ctx.enter_context(tc.tile_pool(name="persist", bufs=1))
            qkvin = ctx.enter_context(tc.tile_pool(name="qkvin", bufs=1))
            work = ctx.enter_context(tc.tile_pool(name="work", bufs=4))
            bcast = ctx.enter_context(tc.tile_pool(name="bcast", bufs=2))
            outsb = ctx.enter_context(tc.tile_pool(name="outsb", bufs=4))
            psS = ctx.enter_context(tc.tile_pool(name="psS", bufs=2, space="PSUM"))
            psQ = ctx.enter_context(tc.tile_pool(name="psQ", bufs=2, space="PSUM"))
            psY = ctx.enter_context(tc.tile_pool(name="psY", bufs=2, space="PSUM"))

            # persistent tensors
            qkT = persist.tile([128, 4, T], BF)           # o-tiles: q01 q23 k01 k23
            vaug = persist.tile([128, NKT, HPC, HD + 1], BF)
            yT = persist.tile([128, 2, T], BF)            # heads stacked on (part, chunk)
            wp_s = persist.tile([128, 2, C], BF)
            m0_s = persist.tile([128, 128], BF)
            bq_s = persist.tile([128, 4], F32)
            bv_s = persist.tile([128, CL], F32)
            ones_f = persist.tile([1, 64], BF)

            PK = QT + 2 * CL + CL      # 1280 packed columns per chunk
            chunk_s = qkvin.tile([128, 8, PK], BF)
            xhi_s = qkvin.tile([128, 3, 8, QT], BF)

            def xw(cc, w):
                """x^T slice [128, QT] for chunk cc, window w."""
                if w == 0:
                    return chunk_s[:, cc, 0:QT]
                return xhi_s[:, w - 1, cc, :]

            wqk_s = chunk_s[:, :, QT:QT + 2 * CL]
            wv_s = chunk_s[:, :, QT + 2 * CL:PK]

            chunk_r = chunkp.rearrange("(cc p) f -> p cc f", p=128)
            xhi_r = xhi.rearrange("(cc p) (w t) -> p cc w t", p=128, t=QT)
            for cc in range(8):
                nc.sync.dma_start(out=chunk_s[:, cc, :], in_=chunk_r[:, cc, :])
            for w in range(3):
                nc.sync.dma_start(out=xhi_s[:, w, :, :], in_=xhi_r[:, :, w, :])
            nc.sync.dma_start(out=wp_s, in_=wpT.rearrange("(cc p) o -> p cc o", p=128))
            # small loads ride the otherwise-idle Act queue
            nc.scalar.dma_start(out=m0_s, in_=m0[:, :])
            nc.scalar.dma_start(out=bq_s, in_=bqk.rearrange("(o p) -> p o", p=128))
            nc.scalar.dma_start(out=bv_s, in_=bvb[:, :])
            nc.scalar.dma_start(out=ones_f, in_=onesf[:, :])
            nc.vector.memset(vaug[:, :, :, HD:HD + 1], 1.0)

            for _rep in range(REPS):
                # ============ phase A: window-0 q/k + v, cc-outer ============
                psA = [psS.tile([128, 2, QT], F32, tag="s", name=f"psA{_i}") for _i in range(2)]
                psV = [psQ.tile([128, QT], F32, tag="q", name=f"psV{_i}") for _i in range(2)]
                for cc in range(8):
                    for o in range(4):
                        nc.tensor.matmul(
                            psA[o // 2][:, o % 2, :],
                            lhsT=wqk_s[:, cc, ts(o, 128)],
                            rhs=xw(cc, 0),
                            start=(cc == 0),
                            stop=(cc == 7),
                        )
                    for tt in range(4):
                        # PSUM start resets the whole bank: only the first
                        # group in a shared bank may assert it; the sibling
                        # half accumulates onto the zeros that reset left.
                        nc.tensor.matmul(
                            psV[tt // 2][:, ts(tt % 2, CL)],
                            lhsT=xw(cc, 0)[:, ts(tt, 128)],
                            rhs=wv_s[:, cc, :],
                            start=(cc == 0 and tt % 2 == 0),
                            stop=(cc == 7),
                            skip_group_check=True,
                        )
                # evict in first-use order (heads 0/1 read o0+o2 first) and
                # split across DVE/Act so the two chains run in parallel.
                for o in (0, 2, 1, 3):
                    sc = 0.125 if o < 2 else 1.0
                    if o >= 2:
                        nc.scalar.activation(
                            out=qkT[:, o, 0:QT],
                            in_=psA[o // 2][:, o % 2, :],
                            func=AF.Identity,
                            scale=sc,
                            bias=bq_s[:, o:o + 1],
                        )
                    else:
                        nc.vector.tensor_scalar(
                            out=qkT[:, o, 0:QT],
                            in0=psA[o // 2][:, o % 2, :],
                            scalar1=sc,
                            scalar2=bq_s[:, o:o + 1],
                            op0=OP.mult,
                            op1=OP.add,
                        )
                for tt in range(4):
                    nc.vector.tensor_add(
                        out=vaug[:, tt, :, 0:HD],
                        in0=psV[tt // 2][:, ts(tt % 2, CL)].rearrange(
                            "p (h d) -> p h d", h=HPC),
                        in1=bv_s.rearrange("p (h d) -> p h d", h=HPC),
                    )

                # ============ filler machinery ============
                crit = deque()   # next window's q/k + v (must flush at boundary)
                lazy = deque()   # c_proj of finished windows

                reserve = [0]   # lazy items held back for the tail

                def pump(n, lazy_ok=False):
                    for _ in range(n):
                        if crit:
                            crit.popleft()()
                        elif lazy_ok and len(lazy) > reserve[0]:
                            lazy.popleft()()
                        else:
                            return

                def flush_crit():
                    while crit:
                        crit.popleft()()

                def enqueue_qkv(jw):
                    t0 = jw * QT

                    def qk_group(o):
                        st = {"ps": None}
                        def mm(cc, st=st, o=o, t0=t0):
                            if cc == 0:
                                st["ps"] = psQ.tile([128, QT], F32, tag="q", name="qkfill")
                            nc.tensor.matmul(
                                st["ps"],
                                lhsT=wqk_s[:, cc, ts(o, 128)],
                                rhs=xw(cc, t0 // QT),
                                start=(cc == 0),
                                stop=(cc == 7),
                            )
                            if cc == 7:
                                sc = 0.125 if o < 2 else 1.0
                                nc.vector.tensor_scalar(
                                    out=qkT[:, o, ds(t0, QT)],
                                    in0=st["ps"],
                                    scalar1=sc,
                                    scalar2=bq_s[:, o:o + 1],
                                    op0=OP.mult,
                                    op1=OP.add,
                                )
                        return mm

                    def add_qk(o):
                        g = qk_group(o)
                        for cc in range(8):
                            crit.append(lambda cc=cc, g=g: g(cc))
                    # heads 0/1 read o0 (q) + o2 (k) first; o1/o3 follow the
                    # v groups so the next window can start before they land
                    add_qk(0)
                    add_qk(2)
                    for tp in range(2):
                        def v_group(tp=tp, jw=jw):
                            st = {"ps": None}
                            def mm(cc, i, st=st, tp=tp, jw=jw):
                                tt = jw * 4 + tp * 2 + i
                                if cc == 0 and i == 0:
                                    st["ps"] = psQ.tile([128, QT], F32, tag="q", name="qkfill")
                                nc.tensor.matmul(
                                    st["ps"][:, ts(i, CL)],
                                    lhsT=xw(cc, tt // 4)[:, ts(tt % 4, 128)],
                                    rhs=wv_s[:, cc, :],
                                    start=(cc == 0 and i == 0),
                                    stop=(cc == 7),
                                    skip_group_check=True,
                                )
                                if cc == 7:
                                    nc.vector.tensor_add(
                                        out=vaug[:, tt, :, 0:HD],
                                        in0=st["ps"][:, ts(i, CL)].rearrange(
                                            "p (h d) -> p h d", h=HPC),
                                        in1=bv_s.rearrange("p (h d) -> p h d", h=HPC),
                                    )
                            return mm
                        g = v_group()
                        for cc in range(8):
                            for i in range(2):
                                crit.append(lambda cc=cc, i=i, g=g: g(cc, i))
                    add_qk(1)
                    add_qk(3)

                def enqueue_cproj(jw):
                    for tl in range(4):
                        tt = jw * 4 + tl
                        def cp_group(tt=tt, jw=jw):
                            st = {"ob": None, "ps": None}
                            def mm(nn_, c2, st=st, tt=tt, jw=jw):
                                if nn_ == 0 and c2 == 0:
                                    st["ob"] = outsb.tile([128, C], BF, tag="ob", name="obt")
                                if c2 == 0:
                                    st["ps"] = psQ.tile([128, QT], F32, tag="q", name="qkfill")
                                nc.tensor.matmul(
                                    st["ps"],
                                    lhsT=yT[:, c2, ts(tt, 128)],
                                    rhs=wp_s[:, c2, ts(nn_, 512)],
                                    start=(c2 == 0),
                                    stop=(c2 == 1),
                                )
                                if c2 == 1:
                                    # alternate evictions DVE/Act so the tail
                                    # is not serialized on one engine
                                    if jw < NQW - 1 or (tt + nn_) % 2 == 0:
                                        nc.vector.tensor_copy(
                                            out=st["ob"][:, ts(nn_, 512)], in_=st["ps"])
                                    else:
                                        nc.scalar.copy(
                                            out=st["ob"][:, ts(nn_, 512)], in_=st["ps"])
                                    nc.sync.dma_start(
                                        out=outp[ts(tt, 128), ts(nn_, 512)],
                                        in_=st["ob"][:, ts(nn_, 512)])
                            return mm
                        g = cp_group()
                        for nn_ in range(2):
                            for c2 in range(2):
                                lazy.append(lambda nn_=nn_, c2=c2, g=g: g(nn_, c2))

                # ============ pipelined windows ============
                for j in range(NQW):
                    # c_proj fillers are held back until the last window,
                    # whose exp stream leaves the PE otherwise underfed.
                    lazy_ok = (j == NQW - 1)
                    if j + 1 < NQW:
                        enqueue_qkv(j + 1)
                    nkt = 4 * (j + 1)
                    for h in range(HPC):
                        # keep a few c_proj fillers for the serial norm tail
                        # of the last head, where nothing else can feed PE
                        reserve[0] = 8 if (j == NQW - 1 and h < HPC - 1) else 0
                        hp, w = h // 2, h % 2
                        pl = 64 * w
                        psy = psY.tile([128, QT], F32, tag="y")
                        pending = []
                        for g in range(nkt // 2):
                            pt = work.tile([128, 2, QT], BF, tag="pt")
                            pss = psS.tile([128, 2, QT], F32, tag="s")
                            cur = []
                            for i in range(2):
                                kt = 2 * g + i
                                m = kt - 4 * j
                                q_lo = m * 128 if m >= 0 else 0
                                n = QT - q_lo
                                nc.tensor.matmul(
                                    pss[:, i, q_lo:QT],
                                    lhsT=qkT[pl:pl + 64, 2 + hp, ts(kt, 128)],
                                    rhs=qkT[pl:pl + 64, hp, ds(j * QT + q_lo, n)],
                                    start=True,
                                    stop=True,
                                )
                                cur.append((pt, i, kt, q_lo))
                            pump(3, lazy_ok)
                            # one exp per group: the [mq, q_lo_i) slice of a
                            # diagonal tile exps stale-but-bounded PSUM data
                            # that the PV matmul never reads.
                            mq = min(q_lo for (_, _, _, q_lo) in cur)
                            nc.scalar.activation(
                                out=pt[:, :, mq:QT], in_=pss[:, :, mq:QT],
                                func=AF.Exp)
                            for (ptile, i, kt, q_lo) in cur:
                                if kt - 4 * j >= 0:
                                    nc.vector.tensor_mul(
                                        out=ptile[:, i, ds(q_lo, 128)],
                                        in0=ptile[:, i, ds(q_lo, 128)],
                                        in1=m0_s,
                                    )
                            pending.append(cur)
                            if len(pending) > 2:
                                for (ptile, i, kt, q_lo) in pending.pop(0):
                                    nc.tensor.matmul(
                                        psy[0:65, q_lo:QT],
                                        lhsT=vaug[:, kt, h, :],
                                        rhs=ptile[:, i, q_lo:QT],
                                        start=(kt == 0),
                                        stop=(kt == nkt - 1),
                                    )
                                pump(1, lazy_ok)
                        for grp in pending:
                            for (ptile, i, kt, q_lo) in grp:
                                nc.tensor.matmul(
                                    psy[0:65, q_lo:QT],
                                    lhsT=vaug[:, kt, h, :],
                                    rhs=ptile[:, i, q_lo:QT],
                                    start=(kt == 0),
                                    stop=(kt == nkt - 1),
                                )
                            pump(1, lazy_ok)
                        # normalize: y^T = y_aug^T * (1/denom); bcast via a
                        # tiny matmul into psy's unused partitions 64-127.
                        # The last head's chain is on the critical tail: split
                        # it into two pipelined q-halves to shorten it.
                        halves = (2 if (j == NQW - 1 and h == HPC - 1) else 1)
                        hw_ = QT // halves
                        rc = bcast.tile([1, QT], BF, tag="rc")
                        dn = bcast.tile([64, QT], BF, tag="dn")
                        for ha in range(halves):
                            qs = ds(ha * hw_, hw_)
                            with nc.allow_low_precision(reason="1/denom bf16"):
                                nc.vector.reciprocal(
                                    out=rc[:, qs], in_=psy[64:65, qs])
                            nc.tensor.matmul(
                                psy[64:128, qs],
                                lhsT=ones_f,
                                rhs=rc[:, qs],
                                start=True,
                                stop=True,
                                skip_group_check=True,
                            )
                            with nc.allow_low_precision(reason="1/denom bf16"):
                                nc.vector.tensor_copy(
                                    out=dn[:, qs], in_=psy[64:128, qs])
                            nc.vector.tensor_mul(
                                out=yT[pl:pl + 64, hp, ds(j * QT + ha * hw_, hw_)],
                                in0=psy[0:64, qs],
                                in1=dn[:, qs],
                            )
                        pump(2, lazy_ok)
                    enqueue_cproj(j)
                    flush_crit()
                while crit or lazy:
                    pump(1, True)
            if DEBUG:
                nc.sync.dma_start(out=dbg_qkT[:, :, :], in_=qkT)
                nc.sync.dma_start(out=dbg_vaug[:, :, :, :], in_=vaug)
                nc.sync.dma_start(out=dbg_yT[:, :, :], in_=yT)

    split_excess_waits(nc)
    return nc


_NC_CACHE = None


def _get_nc():
    global _NC_CACHE
    if _NC_CACHE is None:
        _NC_CACHE = build_nc()
    return _NC_CACHE


def make_in_maps(x, W_attn, b_attn, W_proj):
    bf16 = ml_dtypes.bfloat16
    x = np.asarray(x, np.float32)
    W_attn = np.asarray(W_attn, np.float32)
    b_attn = np.asarray(b_attn, np.float32)
    W_proj = np.asarray(W_proj, np.float32)
    m0 = np.triu(np.ones((128, 128), np.float32))  # keep q >= k
    in_maps = []
    for core in range(NCORES):
        b, g = core // GROUPS, core % GROUPS
        qr = slice(g * CL, (g + 1) * CL)
        kr = slice(C + g * CL, C + (g + 1) * CL)
        vr = slice(2 * C + g * CL, 2 * C + (g + 1) * CL)
        wqk = np.concatenate([W_attn[qr], W_attn[kr]], axis=0)      # [512, 1024]
        xTb = x[b].T                                        # [C, T]
        packed = np.concatenate(
            [xTb[:, :512], wqk.T, W_attn[vr].T], axis=1)    # [C, 1280]
        in_maps.append({
            "chunkp": np.ascontiguousarray(packed).astype(bf16),
            "xhi": np.ascontiguousarray(xTb[:, 512:]).astype(bf16),
            "wpT": np.ascontiguousarray(W_proj[:, g * CL:(g + 1) * CL].T).astype(bf16),
            "bqk": np.concatenate([b_attn[qr] / 8.0, b_attn[kr]]).astype(np.float32),
            "bvb": np.broadcast_to(b_attn[vr], (128, CL)).astype(np.float32).copy(),
            "m0": m0.astype(bf16),
            "onesf": np.ones((1, 64), bf16),
        })
    return in_maps


def kernel(x, W_attn, b_attn, W_proj, b_proj, **_unused):
    nc = _get_nc()
    in_maps = make_in_maps(x, W_attn, b_attn, W_proj)
    res = run_bass_kernel_spmd(nc, in_maps, core_ids=list(range(NCORES)))
    out = np.zeros((B, T, C), np.float32)
    for core in range(NCORES):
        out[core // GROUPS] += res.results[core]["outp"].astype(np.float32)
    out += np.asarray(b_proj, np.float32)[None, None, :]
    return out
